# revision 4
# baseline (speedup 1.0000x reference)
"""Trainium2 Bass kernel for nn_CrossAttention_31078383354530.

Reference computation (b=2, n=m=2048, qd=1024, cd=768, heads=8, dh=128):
    q = x @ Wq; k = ctx @ Wk; v = ctx @ Wv  (split into 8 heads of 128)
    sim = (q @ k^T) * dh**-0.5 over the FLATTENED (b*n)=4096 token axis
    attn = softmax((sim - mean)*1.5 + mean) == softmax(1.5*scale*(q@k^T))
        exactly (the mean-centering is a per-row constant shift)
    out = attn @ v -> merge heads -> y = out @ Wout + bout

Sharding (8 cores): context-token-sharded K/V projection + per-head
AllGather of the bf16 K/V, then each core runs all 8 heads' attention for
its own 512-query-token slice and its own final projection -> the output
is a disjoint row-slice per core (no reduction needed on host).

v2 changes vs the 310us baseline:
  - softmax row-sum is computed OFF the tensor engine: DVE pairwise-tree
    adds of the exp tiles down the ctx-block axis ([128,512] bf16 partials)
    followed by one gpsimd partition_all_reduce per head. This removes the
    256 ones-stationary row-sum matmuls (-26% PE cycles, the dominant
    engine under HAM duty-cycle throttling).
  - head-major K/V projection with one AllGather per head, launched as
    soon as that head's K/V slice is projected (the 8 collectives
    serialize on the ring, so starting the chain ~40us earlier moves the
    whole attention pipeline forward).
  - all input DMAs issued up front; normalization multiplies PV directly
    out of PSUM (drops the pv/rowsum SBUF bounce copies).
"""

import sys

if "/opt/trn_rl_repo" not in sys.path:
    sys.path.insert(0, "/opt/trn_rl_repo")

import ml_dtypes
import numpy as np

import concourse.bass as bass  # noqa: F401
import concourse.mybir as mybir
import concourse.tile as tile
from concourse import bacc, bass_isa, bass_utils

F32 = mybir.dt.float32
BF16 = mybir.dt.bfloat16
AF = mybir.ActivationFunctionType

P = 128
N_CORES = 8
HEADS = 8
DH = 128
TOK = 4096             # b*n flattened token axis (attention mixes batches!)
SLICE = TOK // N_CORES  # 512 tokens per core
QD = 1024
CD = 768
INNER = 1024
KC = QD // P           # 8 qd chunks
CC = CD // P           # 6 cd chunks
JT = TOK // P          # 32 j-tiles per head
GRP = 3                # j-tiles per exp group ([128, 1536] psum, 3 banks)
TT = SLICE // P        # 4 token tiles per core slice
TAU_SCALE = 1.5 * (DH ** -0.5)

_CACHE = {}


def _build():
    nc = bacc.Bacc(num_devices=N_CORES)

    xTs = nc.declare_dram_parameter("xTs", [QD, SLICE], BF16, isOutput=False)
    cTs = nc.declare_dram_parameter("cTs", [CD, SLICE], BF16, isOutput=False)
    Wq = nc.declare_dram_parameter("Wq", [QD, INNER], BF16, isOutput=False)
    Wk = nc.declare_dram_parameter("Wk", [CD, INNER], BF16, isOutput=False)
    Wv = nc.declare_dram_parameter("Wv", [CD, INNER], BF16, isOutput=False)
    Wout = nc.declare_dram_parameter("Wout", [INNER, QD], BF16, isOutput=False)
    boutT = nc.declare_dram_parameter("boutT", [P, KC], F32, isOutput=False)
    yT = nc.declare_dram_parameter("yT", [KC, P, SLICE], F32, isOutput=True)

    with tile.TileContext(nc) as tc:
        with (
            tc.tile_pool(name="const", bufs=1) as const,
            tc.tile_pool(name="sb", bufs=1) as sb,
            tc.tile_pool(name="ps", bufs=1, space="PSUM") as ps,
            tc.tile_pool(name="dram", bufs=1, space="DRAM") as dram,
        ):
            # per-head bounce buffers -> 8 pipelined AllGathers
            kv_in = [dram.tile([2, P, SLICE], BF16, name=f"kv_in{h}")
                     for h in range(HEADS)]
            kv_g = [dram.tile([N_CORES, 2, P, SLICE], BF16,
                              addr_space="Shared", name=f"kv_g{h}")
                    for h in range(HEADS)]

            # tiny warm-up collective issued immediately: absorbs the one-time
            # replica barrier + ring handshake (~35us) behind the projection
            # phase so the first real AllGather starts as soon as K/V head 0
            # is projected
            warm_in = dram.tile([1, 64], BF16, name="warm_in")
            warm_g = dram.tile([N_CORES, 1, 64], BF16, addr_space="Shared",
                               name="warm_g")
            warm_sb = const.tile([1, 64], BF16, name="warm_sb")
            nc.vector.memset(warm_sb[:], 0.0)
            nc.sync.dma_start(warm_in[:], warm_sb[:])
            nc.gpsimd.collective_compute(
                "AllGather", mybir.AluOpType.bypass,
                replica_groups=[list(range(N_CORES))],
                ins=[warm_in.opt()], outs=[warm_g.opt()],
            )

            bout_sb = const.tile([P, KC], F32, name="bout_sb")
            nc.sync.dma_start(bout_sb[:], boutT[:, :])

            # ---- prefetch ALL inputs up front (K/V operands first,
            # interleaved per k-chunk so K head 0 starts ASAP) ----
            cts, wkt, wvt = [], [], []
            for k in range(CC):
                t = sb.tile([P, SLICE], BF16, name=f"cts{k}", tag="cts", bufs=CC)
                nc.sync.dma_start(t[:], cTs[k * P:(k + 1) * P, :])
                cts.append(t)
                t = sb.tile([P, INNER], BF16, name=f"wkt{k}", tag="wkt", bufs=CC)
                nc.sync.dma_start(t[:], Wk[k * P:(k + 1) * P, :])
                wkt.append(t)
                t = sb.tile([P, INNER], BF16, name=f"wvt{k}", tag="wvt", bufs=CC)
                nc.sync.dma_start(t[:], Wv[k * P:(k + 1) * P, :])
                wvt.append(t)
            xts = []
            for k in range(KC):
                t = sb.tile([P, SLICE], BF16, name=f"xts{k}", tag="xts", bufs=KC)
                nc.sync.dma_start(t[:], xTs[k * P:(k + 1) * P, :])
                xts.append(t)
            wqt = []
            for k in range(KC):
                t = sb.tile([P, INNER], BF16, name=f"wqt{k}", tag="wqt", bufs=KC)
                nc.sync.dma_start(t[:], Wq[k * P:(k + 1) * P, :])
                wqt.append(t)
            wo = []
            for cc in range(KC):
                t = sb.tile([P, KC, DH], BF16, name=f"wo{cc}", tag="wo", bufs=KC)
                nc.sync.dma_start(
                    t[:],
                    Wout.ap()[:, cc * DH:(cc + 1) * DH].rearrange(
                        "(k p) c -> p k c", p=P),
                )
                wo.append(t)

            # ---- K/V projection head-major + per-head AllGather ASAP ----
            for h in range(HEADS):
                # K head h: kT [dh, tok] for this ctx slice
                kps = ps.tile([P, GRP * SLICE], F32, name=f"kps{h}", tag="sim",
                              bufs=2)
                for k in range(CC):
                    nc.tensor.matmul(kps[:, :SLICE],
                                     wkt[k][:, h * DH:(h + 1) * DH],
                                     cts[k][:],
                                     start=(k == 0), stop=(k == CC - 1))
                ksb = sb.tile([P, SLICE], BF16, name=f"ksb{h}", tag="ksb", bufs=3)
                nc.vector.tensor_copy(ksb[:], kps[:, :SLICE])
                nc.sync.dma_start(kv_in[h][0], ksb[:])

                # V head h: [tok-in-tile, (tt, dh)] for this ctx slice
                vsb = sb.tile([P, TT, DH], BF16, name=f"vsb{h}", tag="vsb",
                              bufs=3)
                for tt in range(TT):
                    vps = ps.tile([P, GRP * SLICE], F32, name=f"vps{h}_{tt}",
                                  tag="sim", bufs=2)
                    for k in range(CC):
                        nc.tensor.matmul(
                            vps[:, :DH],
                            cts[k][:, tt * P:(tt + 1) * P],
                            wvt[k][:, h * DH:(h + 1) * DH],
                            start=(k == 0), stop=(k == CC - 1))
                    nc.vector.tensor_copy(vsb[:, tt], vps[:, :DH])
                nc.sync.dma_start(kv_in[h][1], vsb[:].rearrange("p t d -> p (t d)"))

                nc.gpsimd.collective_compute(
                    "AllGather", mybir.AluOpType.bypass,
                    replica_groups=[list(range(N_CORES))],
                    ins=[kv_in[h].opt()], outs=[kv_g[h].opt()],
                )

            # ---- Q projection (all heads, own token slice); overlaps AGs ----
            qsb = []
            for m in range(HEADS):
                qps = ps.tile([P, GRP * SLICE], F32, name=f"qps{m}", tag="sim",
                              bufs=2)
                for k in range(KC):
                    nc.tensor.matmul(qps[:, :SLICE],
                                     wqt[k][:, m * DH:(m + 1) * DH],
                                     xts[k][:],
                                     start=(k == 0), stop=(k == KC - 1))
                qt = sb.tile([P, SLICE], BF16, name=f"qsb{m}", tag="qsb",
                             bufs=HEADS)
                nc.vector.tensor_copy(qt[:], qps[:, :SLICE])
                qsb.append(qt)

            # ---- attention, one head at a time over the full 4096 ctx ----
            groups = [list(range(j0, min(j0 + GRP, JT)))
                      for j0 in range(0, JT, GRP)]

            osb = [None] * HEADS
            for h in range(HEADS):
                kh = sb.tile([P, TOK], BF16, name=f"kh{h}", tag="kh", bufs=2)
                vh = sb.tile([P, TOK], BF16, name=f"vh{h}", tag="vh", bufs=2)
                for r in range(N_CORES):
                    nc.sync.dma_start(kh[:, r * SLICE:(r + 1) * SLICE],
                                      kv_g[h][r, 0])
                    nc.sync.dma_start(vh[:, r * SLICE:(r + 1) * SLICE],
                                      kv_g[h][r, 1])
                pv_ps = ps.tile([P, SLICE], F32, name=f"pv{h}", tag="pv", bufs=2)
                gps = []
                for g, js in enumerate(groups):
                    sim_ps = ps.tile([P, GRP * SLICE], F32, name=f"sim{h}_{g}",
                                     tag="sim", bufs=2)
                    for jj, j in enumerate(js):
                        nc.tensor.matmul(
                            sim_ps[:, jj * SLICE:(jj + 1) * SLICE],
                            kh[:, j * P:(j + 1) * P], qsb[h][:],
                            start=True, stop=True)
                    at = sb.tile([P, GRP * SLICE], BF16, name=f"at{h}_{g}",
                                 tag="at", bufs=4)
                    nc.scalar.activation(at[:, :len(js) * SLICE],
                                         sim_ps[:, :len(js) * SLICE], AF.Exp,
                                         scale=TAU_SCALE)
                    for jj, j in enumerate(js):
                        nc.tensor.matmul(pv_ps[:], vh[:, j * P:(j + 1) * P],
                                         at[:, jj * SLICE:(jj + 1) * SLICE],
                                         start=(j == 0), stop=(j == JT - 1))
                    # group partial row-sum on DVE (bf16 2x mode)
                    gp = sb.tile([P, SLICE], BF16, name=f"gp{h}_{g}", tag="gp",
                                 bufs=8)
                    if len(js) == 3:
                        tmp = sb.tile([P, SLICE], BF16, name=f"gt{h}_{g}",
                                      tag="gt", bufs=4)
                        nc.vector.tensor_tensor(tmp[:], at[:, :SLICE],
                                                at[:, SLICE:2 * SLICE],
                                                mybir.AluOpType.add)
                        nc.vector.tensor_tensor(gp[:], tmp[:],
                                                at[:, 2 * SLICE:3 * SLICE],
                                                mybir.AluOpType.add)
                    else:
                        nc.vector.tensor_tensor(gp[:], at[:, :SLICE],
                                                at[:, SLICE:2 * SLICE],
                                                mybir.AluOpType.add)
                    gps.append(gp)
                # accumulate the 11 group partials on gpsimd (f32, paced by
                # group arrival, off the DVE which co-limits the head rate)
                acc = sb.tile([P, SLICE], F32, name=f"acc{h}", tag="acc",
                              bufs=2)
                nc.gpsimd.tensor_tensor(acc[:], gps[0][:], gps[1][:],
                                        mybir.AluOpType.add)
                for g in range(2, len(gps)):
                    nc.gpsimd.tensor_tensor(acc[:], acc[:], gps[g][:],
                                            mybir.AluOpType.add)
                # sum the 128 ctx partitions -> full softmax denominator
                nc.gpsimd.partition_all_reduce(acc[:], acc[:], P,
                                               bass_isa.ReduceOp.add)
                recip = sb.tile([P, SLICE], F32, name=f"recip{h}", tag="recip",
                                bufs=2)
                nc.vector.reciprocal(recip[:], acc[:])
                ot = sb.tile([P, SLICE], BF16, name=f"osb{h}", tag="osb",
                             bufs=HEADS)
                nc.vector.tensor_tensor(ot[:], pv_ps[:], recip[:],
                                        mybir.AluOpType.mult)
                osb[h] = ot

            # ---- final projection: yT[cc] = Wout[:, cc]^T @ out^T + bout ----
            for cc in range(KC):
                yps = ps.tile([P, SLICE], F32, name=f"yps{cc}", tag="pv",
                              bufs=2)
                for ic in range(HEADS):
                    nc.tensor.matmul(yps[:], wo[cc][:, ic], osb[ic][:],
                                     start=(ic == 0), stop=(ic == HEADS - 1))
                yt = sb.tile([P, SLICE], F32, name=f"yt{cc}", tag="yt", bufs=2)
                nc.scalar.activation(yt[:], yps[:], AF.Identity,
                                     bias=bout_sb[:, cc:cc + 1], scale=1.0)
                nc.sync.dma_start(yT.ap()[cc], yt[:])

    nc.compile()
    return nc


def _get_nc():
    if "nc" not in _CACHE:
        _CACHE["nc"] = _build()
    return _CACHE["nc"]


def _bf16(a):
    return np.ascontiguousarray(np.asarray(a, np.float32).astype(ml_dtypes.bfloat16))


def _prep_in_maps(x, context, Wq, Wk, Wv, Wout, bout):
    x_f = np.asarray(x, dtype=np.float32).reshape(TOK, QD)
    c_f = np.asarray(context, dtype=np.float32).reshape(TOK, CD)
    Wq = _bf16(Wq)
    Wk = _bf16(Wk)
    Wv = _bf16(Wv)
    Wout = _bf16(Wout)
    boutT = np.ascontiguousarray(
        np.asarray(bout, dtype=np.float32).reshape(KC, P).T)
    in_maps = []
    for c in range(N_CORES):
        sl = slice(c * SLICE, (c + 1) * SLICE)
        in_maps.append({
            "xTs": _bf16(x_f[sl].T),
            "cTs": _bf16(c_f[sl].T),
            "Wq": Wq, "Wk": Wk, "Wv": Wv, "Wout": Wout, "boutT": boutT,
        })
    return in_maps


def _assemble(results):
    y = np.empty((TOK, QD), dtype=np.float32)
    for c in range(N_CORES):
        yt = results[c]["yT"]   # [KC, P, SLICE]
        y[c * SLICE:(c + 1) * SLICE] = (
            yt.transpose(2, 0, 1).reshape(SLICE, QD))
    return y.reshape(2, TOK // 2, QD)


def run(inputs, trace=False, **kw):
    nc = _get_nc()
    in_maps = _prep_in_maps(**inputs)
    res = bass_utils.run_bass_kernel_spmd(
        nc, in_maps, core_ids=list(range(N_CORES)), trace=trace, **kw)
    return _assemble(res.results), res


def kernel(**inputs):
    out, _ = run(inputs, trace=False)
    return out


# revision 6
# speedup vs baseline: 1.2632x; 1.2632x over previous
"""Trainium2 Bass kernel for nn_CrossAttention_31078383354530.

Reference computation (b=2, n=m=2048, qd=1024, cd=768, heads=8, dh=128):
    q = x @ Wq; k = ctx @ Wk; v = ctx @ Wv  (split into 8 heads of 128)
    sim = (q @ k^T) * dh**-0.5 over the FLATTENED (b*n)=4096 token axis
    attn = softmax((sim - mean)*1.5 + mean) == softmax(1.5*scale*(q@k^T))
        exactly (the mean-centering is a per-row constant shift)
    out = attn @ v -> merge heads -> y = out @ Wout + bout

Sharding (8 cores): context-token-sharded K/V projection + per-head
AllGather of the bf16 K/V, then each core runs all 8 heads' attention for
its own 512-query-token slice and its own final projection -> the output
is a disjoint row-slice per core (no reduction needed on host).

v2 changes vs the 310us baseline:
  - softmax row-sum is computed OFF the tensor engine: DVE pairwise-tree
    adds of the exp tiles down the ctx-block axis ([128,512] bf16 partials)
    followed by one gpsimd partition_all_reduce per head. This removes the
    256 ones-stationary row-sum matmuls (-26% PE cycles, the dominant
    engine under HAM duty-cycle throttling).
  - head-major K/V projection with one AllGather per head, launched as
    soon as that head's K/V slice is projected (the 8 collectives
    serialize on the ring, so starting the chain ~40us earlier moves the
    whole attention pipeline forward).
  - all input DMAs issued up front; normalization multiplies PV directly
    out of PSUM (drops the pv/rowsum SBUF bounce copies).
"""

import sys

if "/opt/trn_rl_repo" not in sys.path:
    sys.path.insert(0, "/opt/trn_rl_repo")

import ml_dtypes
import numpy as np

import concourse.bass as bass  # noqa: F401
import concourse.mybir as mybir
import concourse.tile as tile
from concourse import bacc, bass_isa, bass_utils

F32 = mybir.dt.float32
BF16 = mybir.dt.bfloat16
AF = mybir.ActivationFunctionType

P = 128
N_CORES = 8
HEADS = 8
DH = 128
TOK = 4096             # b*n flattened token axis (attention mixes batches!)
SLICE = TOK // N_CORES  # 512 tokens per core
QD = 1024
CD = 768
INNER = 1024
KC = QD // P           # 8 qd chunks
CC = CD // P           # 6 cd chunks
JT = TOK // P          # 32 j-tiles per head
GRP = 3                # j-tiles per exp group ([128, 1536] psum, 3 banks)
TT = SLICE // P        # 4 token tiles per core slice
TAU_SCALE = 1.5 * (DH ** -0.5)

_CACHE = {}


def _build():
    nc = bacc.Bacc(num_devices=N_CORES)

    xTs = nc.declare_dram_parameter("xTs", [QD, SLICE], BF16, isOutput=False)
    cTs = nc.declare_dram_parameter("cTs", [CD, SLICE], BF16, isOutput=False)
    Wq = nc.declare_dram_parameter("Wq", [QD, INNER], BF16, isOutput=False)
    Wk = nc.declare_dram_parameter("Wk", [CD, INNER], BF16, isOutput=False)
    Wv = nc.declare_dram_parameter("Wv", [CD, INNER], BF16, isOutput=False)
    Wout = nc.declare_dram_parameter("Wout", [INNER, QD], BF16, isOutput=False)
    boutT = nc.declare_dram_parameter("boutT", [P, KC], F32, isOutput=False)
    yT = nc.declare_dram_parameter("yT", [KC, P, SLICE], F32, isOutput=True)

    with tile.TileContext(nc) as tc:
        with (
            tc.tile_pool(name="const", bufs=1) as const,
            tc.tile_pool(name="sb", bufs=1) as sb,
            tc.tile_pool(name="ps", bufs=1, space="PSUM") as ps,
            tc.tile_pool(name="dram", bufs=1, space="DRAM") as dram,
        ):
            # per-head bounce buffers -> 8 pipelined AllGathers
            kv_in = [dram.tile([2, P, SLICE], BF16, name=f"kv_in{h}")
                     for h in range(HEADS)]
            kv_g = [dram.tile([N_CORES, 2, P, SLICE], BF16,
                              addr_space="Shared", name=f"kv_g{h}")
                    for h in range(HEADS)]

            bout_sb = const.tile([P, KC], F32, name="bout_sb")
            nc.sync.dma_start(bout_sb[:], boutT[:, :])

            # ---- prefetch ALL inputs up front (K/V operands first,
            # interleaved per k-chunk so K head 0 starts ASAP) ----
            cts, wkt, wvt = [], [], []
            for k in range(CC):
                t = sb.tile([P, SLICE], BF16, name=f"cts{k}", tag="cts", bufs=CC)
                nc.sync.dma_start(t[:], cTs[k * P:(k + 1) * P, :])
                cts.append(t)
                t = sb.tile([P, INNER], BF16, name=f"wkt{k}", tag="wkt", bufs=CC)
                nc.sync.dma_start(t[:], Wk[k * P:(k + 1) * P, :])
                wkt.append(t)
                t = sb.tile([P, INNER], BF16, name=f"wvt{k}", tag="wvt", bufs=CC)
                nc.sync.dma_start(t[:], Wv[k * P:(k + 1) * P, :])
                wvt.append(t)
            xts = []
            for k in range(KC):
                t = sb.tile([P, SLICE], BF16, name=f"xts{k}", tag="xts", bufs=KC)
                nc.sync.dma_start(t[:], xTs[k * P:(k + 1) * P, :])
                xts.append(t)
            wqt = []
            for k in range(KC):
                t = sb.tile([P, INNER], BF16, name=f"wqt{k}", tag="wqt", bufs=KC)
                nc.sync.dma_start(t[:], Wq[k * P:(k + 1) * P, :])
                wqt.append(t)
            wo = []
            for cc in range(KC):
                t = sb.tile([P, KC, DH], BF16, name=f"wo{cc}", tag="wo", bufs=KC)
                nc.sync.dma_start(
                    t[:],
                    Wout.ap()[:, cc * DH:(cc + 1) * DH].rearrange(
                        "(k p) c -> p k c", p=P),
                )
                wo.append(t)

            # ---- K/V projection head-major + per-head AllGather ASAP ----
            for h in range(HEADS):
                # K head h: kT [dh, tok] for this ctx slice
                kps = ps.tile([P, GRP * SLICE], F32, name=f"kps{h}", tag="sim",
                              bufs=2)
                for k in range(CC):
                    nc.tensor.matmul(kps[:, :SLICE],
                                     wkt[k][:, h * DH:(h + 1) * DH],
                                     cts[k][:],
                                     start=(k == 0), stop=(k == CC - 1))
                ksb = sb.tile([P, SLICE], BF16, name=f"ksb{h}", tag="ksb", bufs=3)
                nc.vector.tensor_copy(ksb[:], kps[:, :SLICE])
                nc.sync.dma_start(kv_in[h][0], ksb[:])

                # V head h: [tok-in-tile, (tt, dh)] for this ctx slice
                vsb = sb.tile([P, TT, DH], BF16, name=f"vsb{h}", tag="vsb",
                              bufs=3)
                for tt in range(TT):
                    vps = ps.tile([P, GRP * SLICE], F32, name=f"vps{h}_{tt}",
                                  tag="sim", bufs=2)
                    for k in range(CC):
                        nc.tensor.matmul(
                            vps[:, :DH],
                            cts[k][:, tt * P:(tt + 1) * P],
                            wvt[k][:, h * DH:(h + 1) * DH],
                            start=(k == 0), stop=(k == CC - 1))
                    nc.vector.tensor_copy(vsb[:, tt], vps[:, :DH])
                nc.sync.dma_start(kv_in[h][1], vsb[:].rearrange("p t d -> p (t d)"))

                nc.gpsimd.collective_compute(
                    "AllGather", mybir.AluOpType.bypass,
                    replica_groups=[list(range(N_CORES))],
                    ins=[kv_in[h].opt()], outs=[kv_g[h].opt()],
                )

            # ---- Q projection (all heads, own token slice); overlaps AGs ----
            qsb = []
            for m in range(HEADS):
                qps = ps.tile([P, GRP * SLICE], F32, name=f"qps{m}", tag="sim",
                              bufs=2)
                for k in range(KC):
                    nc.tensor.matmul(qps[:, :SLICE],
                                     wqt[k][:, m * DH:(m + 1) * DH],
                                     xts[k][:],
                                     start=(k == 0), stop=(k == KC - 1))
                qt = sb.tile([P, SLICE], BF16, name=f"qsb{m}", tag="qsb",
                             bufs=HEADS)
                nc.vector.tensor_copy(qt[:], qps[:, :SLICE])
                qsb.append(qt)

            # ---- attention, one head at a time over the full 4096 ctx ----
            groups = [list(range(j0, min(j0 + GRP, JT)))
                      for j0 in range(0, JT, GRP)]

            osb = [None] * HEADS
            for h in range(HEADS):
                kh = sb.tile([P, TOK], BF16, name=f"kh{h}", tag="kh", bufs=2)
                vh = sb.tile([P, TOK], BF16, name=f"vh{h}", tag="vh", bufs=2)
                for r in range(N_CORES):
                    nc.sync.dma_start(kh[:, r * SLICE:(r + 1) * SLICE],
                                      kv_g[h][r, 0])
                    nc.sync.dma_start(vh[:, r * SLICE:(r + 1) * SLICE],
                                      kv_g[h][r, 1])
                pv_ps = ps.tile([P, SLICE], F32, name=f"pv{h}", tag="pv", bufs=2)
                ats = []
                for g, js in enumerate(groups):
                    sim_ps = ps.tile([P, GRP * SLICE], F32, name=f"sim{h}_{g}",
                                     tag="sim", bufs=2)
                    for jj, j in enumerate(js):
                        nc.tensor.matmul(
                            sim_ps[:, jj * SLICE:(jj + 1) * SLICE],
                            kh[:, j * P:(j + 1) * P], qsb[h][:],
                            start=True, stop=True)
                    at = sb.tile([P, GRP * SLICE], BF16, name=f"at{h}_{g}",
                                 tag="at", bufs=4)
                    nc.scalar.activation(at[:, :len(js) * SLICE],
                                         sim_ps[:, :len(js) * SLICE], AF.Exp,
                                         scale=TAU_SCALE)
                    if len(js) < GRP:   # pad short group so the tree is uniform
                        nc.vector.memset(at[:, len(js) * SLICE:], 0.0)
                    for jj, j in enumerate(js):
                        nc.tensor.matmul(pv_ps[:], vh[:, j * P:(j + 1) * P],
                                         at[:, jj * SLICE:(jj + 1) * SLICE],
                                         start=(j == 0), stop=(j == JT - 1))
                    ats.append(at)
                # row-sum on DVE: pairwise tree of WIDE [128, 1536] adds over
                # the 11 exp tiles (3 independent partial columns ride along),
                # then fold the 3 columns to [128, 512]
                lvl = ats
                li = 0
                while len(lvl) > 1:
                    nxt = []
                    i = 0
                    while i + 1 < len(lvl):
                        t = sb.tile([P, GRP * SLICE], BF16,
                                    name=f"tr{h}_{li}_{i}", tag="tr", bufs=6)
                        nc.vector.tensor_tensor(t[:], lvl[i][:], lvl[i + 1][:],
                                                mybir.AluOpType.add)
                        nxt.append(t)
                        i += 2
                    if i < len(lvl):
                        nxt.append(lvl[i])
                    lvl = nxt
                    li += 1
                root = lvl[0]
                fold = sb.tile([P, SLICE], BF16, name=f"fold{h}", tag="fold",
                               bufs=2)
                nc.vector.tensor_tensor(fold[:], root[:, :SLICE],
                                        root[:, SLICE:2 * SLICE],
                                        mybir.AluOpType.add)
                rs_sum = sb.tile([P, SLICE], BF16, name=f"rsum{h}", tag="rsum",
                                 bufs=2)
                nc.vector.tensor_tensor(rs_sum[:], fold[:],
                                        root[:, 2 * SLICE:3 * SLICE],
                                        mybir.AluOpType.add)
                # sum the 128 ctx partitions -> full softmax denominator
                rs_all = sb.tile([P, SLICE], F32, name=f"rs{h}", tag="rs",
                                 bufs=2)
                nc.gpsimd.partition_all_reduce(rs_all[:], rs_sum[:], P,
                                               bass_isa.ReduceOp.add)
                recip = sb.tile([P, SLICE], F32, name=f"recip{h}", tag="recip",
                                bufs=2)
                nc.vector.reciprocal(recip[:], rs_all[:])
                ot = sb.tile([P, SLICE], BF16, name=f"osb{h}", tag="osb",
                             bufs=HEADS)
                nc.vector.tensor_tensor(ot[:], pv_ps[:], recip[:],
                                        mybir.AluOpType.mult)
                osb[h] = ot

            # ---- final projection: yT[cc] = Wout[:, cc]^T @ out^T + bout ----
            for cc in range(KC):
                yps = ps.tile([P, SLICE], F32, name=f"yps{cc}", tag="pv",
                              bufs=2)
                for ic in range(HEADS):
                    nc.tensor.matmul(yps[:], wo[cc][:, ic], osb[ic][:],
                                     start=(ic == 0), stop=(ic == HEADS - 1))
                yt = sb.tile([P, SLICE], F32, name=f"yt{cc}", tag="yt", bufs=2)
                nc.scalar.activation(yt[:], yps[:], AF.Identity,
                                     bias=bout_sb[:, cc:cc + 1], scale=1.0)
                nc.sync.dma_start(yT.ap()[cc], yt[:])

    nc.compile()
    return nc


def _get_nc():
    if "nc" not in _CACHE:
        _CACHE["nc"] = _build()
    return _CACHE["nc"]


def _bf16(a):
    return np.ascontiguousarray(np.asarray(a, np.float32).astype(ml_dtypes.bfloat16))


def _prep_in_maps(x, context, Wq, Wk, Wv, Wout, bout):
    x_f = np.asarray(x, dtype=np.float32).reshape(TOK, QD)
    c_f = np.asarray(context, dtype=np.float32).reshape(TOK, CD)
    Wq = _bf16(Wq)
    Wk = _bf16(Wk)
    Wv = _bf16(Wv)
    Wout = _bf16(Wout)
    boutT = np.ascontiguousarray(
        np.asarray(bout, dtype=np.float32).reshape(KC, P).T)
    in_maps = []
    for c in range(N_CORES):
        sl = slice(c * SLICE, (c + 1) * SLICE)
        in_maps.append({
            "xTs": _bf16(x_f[sl].T),
            "cTs": _bf16(c_f[sl].T),
            "Wq": Wq, "Wk": Wk, "Wv": Wv, "Wout": Wout, "boutT": boutT,
        })
    return in_maps


def _assemble(results):
    y = np.empty((TOK, QD), dtype=np.float32)
    for c in range(N_CORES):
        yt = results[c]["yT"]   # [KC, P, SLICE]
        y[c * SLICE:(c + 1) * SLICE] = (
            yt.transpose(2, 0, 1).reshape(SLICE, QD))
    return y.reshape(2, TOK // 2, QD)


def run(inputs, trace=False, **kw):
    nc = _get_nc()
    in_maps = _prep_in_maps(**inputs)
    res = bass_utils.run_bass_kernel_spmd(
        nc, in_maps, core_ids=list(range(N_CORES)), trace=trace, **kw)
    return _assemble(res.results), res


def kernel(**inputs):
    out, _ = run(inputs, trace=False)
    return out
